# revision 13
# baseline (speedup 1.0000x reference)
"""MiniCausalAttention on 8 NeuronCores (Trainium2, Bass/Tile).

Problem: x[4,2048,1024] fp32; q/k/v = x@w+b; causal softmax(q k^T/sqrt(D)) @ v.

Sharding: 8 cores = (batch b in 0..3) x (half h in 0..1). Core (b,h) handles
query tiles t' = 2t+h for t in 0..7 (interleaved 128-row tiles), so every
core sees the SAME set of causal key-extents nk(t) = 256*(t+1) -> one SPMD
program, perfectly balanced.

Projection reassociation (exact algebra, host-precomputed M = Wq Wk^T and
u = Wk bq):
  scores  S = q k^T = x_q M x^T + 1 (x) (x u)^T  (+ per-query terms that
          cancel in softmax and are dropped)
  output  O = P_norm v = [(P x) Wv] / rowsum + bv
so neither K nor V is ever materialized: phase A computes only
G^T = (x_q M)^T and the key-bias row c' = (x u)^T; the c' term is folded
into the S accumulation as a K=1 matmul of ones^T (x) c'.

All matmuls run in bf16 (full PE rate); PSUM accumulation fp32; softmax
statistics fp32. G^T/x^T are [d_model partition, token free] so S comes out
query-major; x rows are token-major for Z = P x after a PE transpose of each
128x128 P tile; Z is PE-transposed again for O = Z Wv.
"""

import sys

if "/opt/trn_rl_repo" not in sys.path:
    sys.path.insert(0, "/opt/trn_rl_repo")

import numpy as np
import ml_dtypes

import concourse.bass as bass  # noqa: F401
import concourse.tile as tile
from concourse import bacc, mybir
from concourse.bass_utils import run_bass_kernel_spmd
from concourse.masks import make_identity

BF16 = mybir.dt.bfloat16
F32 = mybir.dt.float32
AF = mybir.ActivationFunctionType

B, L, D = 4, 2048, 1024
P = 128
NQT = 8          # q-tiles per core, 128 rows each
SCALE = 1.0 / 32.0   # 1/sqrt(D)
NEG = -1.0e30

_CACHED = {}


def build_nc():
    nc = bacc.Bacc(None, target_bir_lowering=False)

    xt = nc.declare_dram_parameter("xt", [D, L], BF16, isOutput=False)    # x^T
    xr = nc.declare_dram_parameter("xr", [L, D], BF16, isOutput=False)    # x rows
    xtq = nc.declare_dram_parameter("xtq", [D, D], BF16, isOutput=False)  # q cols of x^T
    mm_w = nc.declare_dram_parameter("mm_w", [D, D], BF16, isOutput=False)  # Wq Wk^T
    wv = nc.declare_dram_parameter("wv", [D, D], BF16, isOutput=False)
    um = nc.declare_dram_parameter("um", [P, 8], BF16, isOutput=False)    # Wk bq
    bvr = nc.declare_dram_parameter("bvr", [1, D], BF16, isOutput=False)
    mask = nc.declare_dram_parameter("mask", [P, 256], F32, isOutput=False)
    out = nc.declare_dram_parameter("out", [D, D], F32, isOutput=True)

    with tile.TileContext(nc) as tc:
        with tc.tile_pool(name="persist", bufs=1) as persist:
            xt_sb = persist.tile([P, 8, L], BF16)    # x^T: [d-part, ct, token]
            xr_sb = persist.tile([P, 16, D], BF16)   # x: [tok-part, tt, d]
            gt_sb = persist.tile([P, 8, D], BF16)    # G^T: [d-part, dt, qcol]
            xtq_sb = persist.tile([P, 8, D], BF16)
            m_sb = persist.tile([P, 8, D], BF16)
            wv_sb = persist.tile([P, 8, D], BF16)
            um_sb = persist.tile([P, 8], BF16)
            cx_sb = persist.tile([1, L], BF16)       # c' = (x u)^T key-bias row
            bvr_sb = persist.tile([1, D], BF16)
            mask_sb = persist.tile([P, 256], F32)
            ident = persist.tile([P, P], BF16)
            ones_sb = persist.tile([1, P], BF16)
            bvb_sb = persist.tile([P, D], F32)       # broadcast bias 1 (x) bv

            nc.sync.dma_start(out=um_sb, in_=um[:, :])
            nc.sync.dma_start(out=bvr_sb, in_=bvr[:, :])
            nc.sync.dma_start(out=mask_sb, in_=mask[:, :])
            make_identity(nc, ident)
            nc.vector.memset(ones_sb, 1.0)

            # input streams, roughly in first-use order
            for i in range(8):
                nc.sync.dma_start(out=xt_sb[:, i, :512], in_=xt[i * P:(i + 1) * P, :512])
            for i in range(8):
                nc.sync.dma_start(out=m_sb[:, i, :], in_=mm_w[i * P:(i + 1) * P, :])
            for i in range(8):
                nc.sync.dma_start(out=xtq_sb[:, i, :], in_=xtq[i * P:(i + 1) * P, :])
            for c in range(1, 4):
                for i in range(8):
                    nc.sync.dma_start(out=xt_sb[:, i, c * 512:(c + 1) * 512],
                                      in_=xt[i * P:(i + 1) * P, c * 512:(c + 1) * 512])
            # xr/wv go on the gpsimd (SWDGE) queue: parallel to the sync
            # stream, they are needed only once phase B reaches Z/O.
            for tt in range(16):
                nc.gpsimd.dma_start(out=xr_sb[:, tt, :],
                                    in_=xr[tt * P:(tt + 1) * P, :])
            for i in range(8):
                nc.gpsimd.dma_start(out=wv_sb[:, i, :], in_=wv[i * P:(i + 1) * P, :])

            # Two PSUM pools spanning phases A and B (8 banks total):
            # psA: tags s (x2) + ptp/pc (x2); psC: pz0, pz1, po0, po1.
            with tc.tile_pool(name="bwork", bufs=2) as bwork, \
                 tc.tile_pool(name="psA", bufs=2, space="PSUM") as psS, \
                 tc.tile_pool(name="psC", bufs=1, space="PSUM") as psZ:
                psT = psS   # transposes + c' share the psA pool (tag ptp)
                psO = psZ

                # bvb = 1 (x) bv, built once via a K=1 matmul. The repeats
                # are HAM warmup: ~8 us of back-to-back PE work spanning the
                # input-DMA wait so G starts at 2.4 GHz instead of 1.2.
                for dc in range(2):
                    pb = psZ.tile([P, 512], F32, tag=f"po{dc}", name=f"pb{dc}")
                    for rep in range(18):
                        nc.tensor.matmul(pb, ones_sb,
                                         bvr_sb[:, dc * 512:(dc + 1) * 512],
                                         start=True, stop=True)
                    nc.scalar.copy(bvb_sb[:, dc * 512:(dc + 1) * 512], pb)

                # ---------- Phase A: c' = (x u)^T and G^T = (x_q M)^T ------
                # c' chunk 0 only needs the first 1 MB of x^T: earliest PE
                # work. G runs qc-outer so B's first tiles unblock early.
                def cprime_chunk(c):
                    pc = psS.tile([1, 512], F32, tag="ptp", name=f"pc{c}")
                    for ct in range(8):
                        nc.tensor.matmul(
                            pc,
                            um_sb[:, ct:ct + 1],
                            xt_sb[:, ct, c * 512:(c + 1) * 512],
                            start=(ct == 0),
                            stop=(ct == 7),
                        )
                    nc.scalar.copy(cx_sb[:, c * 512:(c + 1) * 512], pc)

                cprime_chunk(0)
                for qc in range(2):
                    for dt in range(8):
                        pg = psS.tile([P, 512], F32, tag="s", name="pg")
                        for ct in range(8):
                            nc.tensor.matmul(
                                pg,
                                m_sb[:, ct, dt * P:(dt + 1) * P],
                                xtq_sb[:, ct, qc * 512:(qc + 1) * 512],
                                start=(ct == 0),
                                stop=(ct == 7),
                            )
                        nc.scalar.copy(gt_sb[:, dt, qc * 512:(qc + 1) * 512], pg)
                    if qc == 0:
                        for c in range(1, 4):
                            cprime_chunk(c)

                # ------------- Phase B: attention per q-tile ---------------
                # S chunks (+ c' fold-in) -> exp (+rowsum) -> P^T -> Z = P x
                # -> Z^T -> O = Z^T.T Wv, scaled by 1/rowsum, + bvb.
                for t in range(NQT):
                    nk = 256 * (t + 1)
                    nkc = (nk + 511) // 512  # 512-chunks (last may be 256)

                    p_sb = bwork.tile([P, 2048], BF16, tag="p")
                    rsum = bwork.tile([P, 4], F32, tag="rsum")
                    pz = [psZ.tile([P, 512], F32, tag=f"pz{dc}", name=f"pz{dc}")
                          for dc in range(2)]

                    for c in range(nkc):
                        w = min(512, nk - c * 512)
                        ps = psS.tile([P, 512], F32, tag="s")
                        for dt in range(8):
                            nc.tensor.matmul(
                                ps[:, :w],
                                gt_sb[:, dt, t * P:(t + 1) * P],
                                xt_sb[:, dt, c * 512:c * 512 + w],
                                start=(dt == 0),
                                stop=False,
                            )
                        # key-bias c' via K=1 matmul of ones^T (x) cx
                        nc.tensor.matmul(
                            ps[:, :w],
                            ones_sb,
                            cx_sb[:, c * 512:c * 512 + w],
                            start=False,
                            stop=True,
                        )
                        if c == nkc - 1:
                            nc.vector.tensor_add(ps[:, w - 256:w],
                                                 ps[:, w - 256:w], mask_sb)
                        nc.scalar.activation(
                            p_sb[:, c * 512:c * 512 + w], ps[:, :w], AF.Exp,
                            scale=SCALE, accum_out=rsum[:, c:c + 1])
                        for kt in range(c * 4, c * 4 + w // P):
                            ptp = psT.tile([P, P], BF16, tag="ptp")
                            nc.tensor.transpose(
                                ptp, p_sb[:, kt * P:(kt + 1) * P], ident)
                            pt_sb = bwork.tile([P, P], BF16, tag="pt")
                            nc.vector.tensor_copy(pt_sb, ptp)
                            for dc in range(2):
                                nc.tensor.matmul(
                                    pz[dc],
                                    pt_sb,
                                    xr_sb[:, kt, dc * 512:(dc + 1) * 512],
                                    start=(kt == 0),
                                    stop=(kt == nk // P - 1),
                                )

                    # Z -> SBUF (bf16), then Z^T tiles, then O = Z Wv
                    z_sb = bwork.tile([P, D], BF16, tag="z")
                    for dc in range(2):
                        nc.scalar.copy(z_sb[:, dc * 512:(dc + 1) * 512], pz[dc])
                    po = [psO.tile([P, 512], F32, tag=f"po{dc}", name=f"po{dc}")
                          for dc in range(2)]
                    for cc in range(8):
                        ztp = psT.tile([P, P], BF16, tag="ptp", name="ztp")
                        nc.tensor.transpose(ztp, z_sb[:, cc * P:(cc + 1) * P], ident)
                        zt_sb = bwork.tile([P, P], BF16, tag="zt")
                        nc.vector.tensor_copy(zt_sb, ztp)
                        for dc in range(2):
                            nc.tensor.matmul(
                                po[dc],
                                zt_sb,
                                wv_sb[:, cc, dc * 512:(dc + 1) * 512],
                                start=(cc == 0),
                                stop=(cc == 7),
                            )

                    rinv = bwork.tile([P, 1], F32, tag="rinv")
                    rtot = bwork.tile([P, 1], F32, tag="rtot")
                    nc.vector.reduce_sum(rtot, rsum[:, :nkc], axis=mybir.AxisListType.X)
                    nc.vector.reciprocal(rinv, rtot)
                    o_sb = bwork.tile([P, D], F32, tag="o")
                    for dc in range(2):
                        sl = slice(dc * 512, (dc + 1) * 512)
                        nc.scalar.activation(o_sb[:, sl], po[dc], AF.Copy,
                                             scale=rinv)
                        nc.vector.tensor_add(o_sb[:, sl], o_sb[:, sl],
                                             bvb_sb[:, sl])
                    nc.sync.dma_start(out=out[t * P:(t + 1) * P, :], in_=o_sb)

    nc.finalize()
    return nc


def _prep_inputs(x, wq, bq, wk, bk, wv, bv):
    bf = ml_dtypes.bfloat16
    wq32 = np.asarray(wq, np.float32)
    wk32 = np.asarray(wk, np.float32)
    m_host = (wq32 @ wk32.T).astype(bf)                 # Wq Wk^T
    u_host = (wk32 @ np.asarray(bq, np.float32))        # Wk bq, [D]
    um = np.ascontiguousarray(u_host.reshape(8, P).T).astype(bf)
    wv_b = np.ascontiguousarray(wv, np.float32).astype(bf)
    bvr = np.asarray(bv, np.float32).reshape(1, D).astype(bf)

    i = np.arange(P)[:, None]
    j = np.arange(256)[None, :]
    masks = [np.where(j <= i + P * h, 0.0, NEG).astype(np.float32)
             for h in range(2)]

    in_maps = []
    for core in range(8):
        b, h = core // 2, core % 2
        xb = np.asarray(x[b], np.float32)
        xT = np.ascontiguousarray(xb.T).astype(bf)
        xR = xb.astype(bf)
        qcols = (np.arange(8)[:, None] * 2 + h) * P + np.arange(P)[None, :]
        xTq = np.ascontiguousarray(xT[:, qcols.ravel()])
        in_maps.append({
            "xt": xT, "xr": xR, "xtq": xTq, "mm_w": m_host, "wv": wv_b,
            "um": um, "bvr": bvr, "mask": masks[h],
        })
    return in_maps


def kernel(x, wq, bq, wk, bk, wv, bv, _trace=False, _trace_kwargs=None):
    if "nc" not in _CACHED:
        _CACHED["nc"] = build_nc()
    nc = _CACHED["nc"]
    in_maps = _prep_inputs(x, wq, bq, wk, bk, wv, bv)
    kw = {}
    if _trace:
        kw = dict(trace=True, **(_trace_kwargs or {}))
    res = run_bass_kernel_spmd(nc, in_maps, list(range(8)), **kw)
    out = np.empty((B, L, D), np.float32)
    for core in range(8):
        b, h = core // 2, core % 2
        o = np.asarray(res.results[core]["out"], np.float32)
        out[b].reshape(16, P, D)[h::2] = o.reshape(NQT, P, D)
    if _trace:
        _CACHED["last_results"] = res
    return out
